# revision 26
# baseline (speedup 1.0000x reference)
"""CTC model kernel for Trainium2 (8 NeuronCores, pure data parallel).

reference computation:
    logits = softmax(feat_vec @ W + b)          # [1024, 80, 78]
    best   = argmax(logits, -1)                 # [1024, 80]
    res    = ctc_greedy_decode(best)            # [1024, 30] int32 (pad -1)
    return logits, res

Sharding: batch 1024 -> 8 cores x 128 rows. W/b replicated. Decode is
per-row so it shards with batch.

Per-core dataflow (B=128, T=80, D=512, C=78):
  16 groups of 5 timesteps:
    DMA feat[:, 5g:5g+5, :]  (10KB/partition, contiguous)
    per t: 4x PE transpose [128b,128d]->[128d,128b] (PSUM),
           PSUM->SBUF copy (ACT/DVE), 4x fp32 matmul vs W chunks -> PSUM
    ACT exp -> SBUF; DVE row-sums + reciprocal + normalize -> DMA out
    per t: DVE max8 + max_index -> best indices (argmax)
  decode tail (DVE): collapse repeats / drop blanks via prefix-scan cumsum
  and a one-hot gather, -> [128, 30] int32 -> DMA out.
"""
import sys

sys.path.insert(0, "/opt/trn_rl_repo")

import numpy as np

import concourse.bass as bass  # noqa: F401  (engine namespaces)
import concourse.mybir as mybir
import concourse.tile as tile
from concourse import bacc
from concourse.bass_utils import run_bass_kernel_spmd

F32 = mybir.dt.float32
I32 = mybir.dt.int32
I16 = mybir.dt.int16
U32 = mybir.dt.uint32
AX = mybir.AxisListType
ALU = mybir.AluOpType
ACTF = mybir.ActivationFunctionType

B_FULL, T, D, C = 1024, 80, 512, 78
NCORES = 8
BS = B_FULL // NCORES          # 128 rows per core
GT = 5                         # timesteps per group
NG = T // GT                   # 16 groups
NCH = D // 128                 # 4 contraction chunks
PRED = 30
BLANK = 77

TRACE = False                  # test.py sets kernel.TRACE = True to profile
TRACE_KWARGS = {}
LAST_RESULT = None

_NC_CACHE = {}

QT = T // 4  # decode piece length
NPIECE = 4


def _decode_piece(nc, dec, best8, q):
    """CTC decode for timesteps [q*QT, (q+1)*QT) — all int32 on DVE.
    Pieces 0..2 run mid-kernel (hidden under later groups); piece 3 is the
    only decode work on the tail. Cumsum state chains across pieces."""
    ALU = mybir.AluOpType
    AX = mybir.AxisListType
    ta, tb = q * QT, (q + 1) * QT
    best_i, prevb, keep, cum, posm, zeros = (
        dec["best_i"], dec["prevb"], dec["keep"], dec["cum"], dec["posm"],
        dec["zeros"])
    nc.vector.tensor_copy(best_i[:, ta:tb], best8[:, ta:tb, 0].bitcast(I32))
    if q == 0:
        nc.vector.memset(prevb[:, 0:1], -1)
        nc.vector.tensor_copy(prevb[:, 1:tb], best_i[:, 0:tb - 1])
    else:
        nc.vector.tensor_copy(prevb[:, ta:tb], best_i[:, ta - 1:tb - 1])
    n1 = dec["n1"]
    nc.vector.tensor_tensor(n1[:, ta:tb], best_i[:, ta:tb], prevb[:, ta:tb],
                            op=ALU.not_equal)
    n2 = dec["n2"]
    nc.vector.tensor_scalar(n2[:, ta:tb], best_i[:, ta:tb], BLANK, None,
                            op0=ALU.not_equal)
    nc.vector.tensor_tensor(keep[:, ta:tb], n1[:, ta:tb], n2[:, ta:tb],
                            op=ALU.mult)
    init = 0.0 if q == 0 else cum[:, ta - 1:ta]
    nc.vector.tensor_tensor_scan(cum[:, ta:tb], keep[:, ta:tb],
                                 zeros[:, ta:tb], init,
                                 op0=ALU.add, op1=ALU.add)
    pos = dec["pos"]
    nc.vector.tensor_scalar(pos[:, ta:tb], cum[:, ta:tb], -1, None,
                            op0=ALU.add)
    nc.vector.memset(posm[:, ta:tb], 999)
    nc.vector.copy_predicated(posm[:, ta:tb], keep[:, ta:tb], pos[:, ta:tb])
    bestp1 = dec["bestp1"]
    nc.vector.tensor_scalar(bestp1[:, ta:tb], best_i[:, ta:tb], 1, None,
                            op0=ALU.add)

    iota3v = dec["iota3"][:].rearrange("p (a b) -> p a b", a=PRED)
    m3v = dec["mask3"][:].rearrange("p (a b) -> p a b", a=PRED)
    nc.vector.tensor_tensor(
        m3v,
        posm[:, ta:tb].unsqueeze(1).broadcast_to([128, PRED, QT]),
        iota3v[:, :, ta:tb],
        op=ALU.is_equal)
    t3v = dec["tokm"][:].rearrange("p (a b) -> p a b", a=PRED)
    nc.vector.tensor_tensor(
        t3v, m3v,
        bestp1[:, ta:tb].unsqueeze(1).broadcast_to([128, PRED, QT]),
        op=ALU.mult)
    nc.vector.tensor_reduce(dec["res30h"][:, q, :], t3v, axis=AX.X,
                            op=ALU.max)


def _build(b_nonzero: bool):
    nc = bacc.Bacc(None, target_bir_lowering=False)

    feat_d = nc.dram_tensor("feat", [BS, T, D], F32, kind="ExternalInput")
    w_d = nc.dram_tensor("w", [D, C], F32, kind="ExternalInput")
    id_d = nc.dram_tensor("ident", [128, 128], F32, kind="ExternalInput")
    if b_nonzero:
        eb_d = nc.dram_tensor("eb", [1, C], F32, kind="ExternalInput")
    logits_d = nc.dram_tensor("logits", [BS, T, C], F32, kind="ExternalOutput")
    res_d = nc.dram_tensor("res", [BS, PRED], I32, kind="ExternalOutput")

    with tile.TileContext(nc) as tc:
        with tc.tile_pool(name="const", bufs=1) as constp, \
             tc.tile_pool(name="featp", bufs=4) as featp, \
             tc.tile_pool(name="ftsp", bufs=6) as ftsp, \
             tc.tile_pool(name="ep", bufs=3) as ep, \
             tc.tile_pool(name="probsp", bufs=2) as probsp, \
             tc.tile_pool(name="smallp", bufs=2) as smallp, \
             tc.tile_pool(name="argp", bufs=1) as argp, \
             tc.tile_pool(name="decp", bufs=1) as decp, \
             tc.tile_pool(name="ftpp", bufs=4, space="PSUM") as ftpp, \
             tc.tile_pool(name="lgp", bufs=3, space="PSUM") as lgp:

            # ---- constants (scalar-engine DGE queue: overlaps feat triggers) ----
            id_s = constp.tile([128, 128], F32, tag="id_s")
            nc.sync.dma_start(id_s[:], id_d[:])
            w_s = constp.tile([128, NCH, C], F32, tag="w_s")
            nc.scalar.dma_start(w_s[:], w_d[:].rearrange("(c p) n -> p c n", p=128))
            if b_nonzero:
                eb_s = constp.tile([128, C], F32, tag="eb_s")
                nc.scalar.dma_start(eb_s[:], eb_d[:].partition_broadcast(128))

            # iota over the middle axis of [128, PRED, T]: value = j
            iota3 = decp.tile([128, PRED * T], I16, tag="iota3")
            nc.gpsimd.iota(iota3[:].rearrange("p (a b) -> p a b", a=PRED),
                           pattern=[[1, PRED], [0, T]], base=0,
                           channel_multiplier=0)

            # persistent decode state ([128, T] int32 each)
            dec = {"iota3": iota3}
            for nm in ["best_i", "prevb", "keep", "cum", "posm", "zeros",
                       "n1", "n2", "pos", "bestp1"]:
                dec[nm] = decp.tile([128, T], I16, tag=f"dec_{nm}", name=f"dec_{nm}")
            dec["mask3"] = decp.tile([128, PRED * QT], I16, tag="dec_mask3", name="dec_mask3")
            dec["tokm"] = decp.tile([128, PRED * QT], I16, tag="dec_tokm", name="dec_tokm")
            dec["res30h"] = decp.tile([128, NPIECE, PRED], I16, tag="dec_res30h", name="dec_res30h")
            nc.vector.memset(dec["zeros"][:], 0)

            # per-t argmax results (free layout [t, 8])
            maxv = argp.tile([128, T, 8], F32, tag="maxv")
            best8 = argp.tile([128, T, 8], U32, tag="best8")

            # HAM warm-up: dummy transposes while the first feat DMA lands
            # (PE runs at 1.2 GHz until ~3.4us of sustained activity)
            warmp = ftpp.tile([128, 128], F32, tag="warm", name="warm", bufs=1)
            for _ in range(12):
                nc.tensor.transpose(warmp[:], id_s[:], id_s[:])

            # ---- main loop over groups of GT timesteps ----
            for g in range(NG):
                t0 = g * GT
                ftile = featp.tile([128, GT, D], F32, tag="ftile")
                if g == 0:
                    # split the first load so PE can start ~3us earlier
                    for j in range(GT):
                        nc.sync.dma_start(ftile[:, j, :], feat_d[:, j, :])
                else:
                    nc.sync.dma_start(ftile[:], feat_d[:, t0:t0 + GT, :])

                lg = lgp.tile([128, GT, C], F32, tag="lg")
                for j in range(GT):
                    ftp = ftpp.tile([128, D], F32, tag="ftp")
                    for c in range(NCH):
                        nc.tensor.transpose(
                            ftp[:, c * 128:(c + 1) * 128],
                            ftile[:, j, c * 128:(c + 1) * 128],
                            id_s[:])
                    fts = ftsp.tile([128, D], F32, tag="fts")
                    nc.scalar.copy(fts[:], ftp[:])
                    for c in range(NCH):
                        nc.tensor.matmul(
                            lg[:, j, :],
                            fts[:, c * 128:(c + 1) * 128],
                            w_s[:, c, :],
                            start=(c == 0), stop=(c == NCH - 1))

                e = ep.tile([128, GT, C], F32, tag="e")
                if g == NG - 1:
                    # split exp per-t so the final argmax chain starts sooner
                    for j in reversed(range(GT)):
                        nc.scalar.activation(e[:, j, :], lg[:, j, :], ACTF.Exp)
                else:
                    nc.scalar.activation(e[:], lg[:], ACTF.Exp)
                if b_nonzero:
                    nc.vector.tensor_tensor(
                        e[:], e[:],
                        eb_s[:].unsqueeze(1).broadcast_to([128, GT, C]),
                        op=ALU.mult)

                # argmax first: it gates the decode tail
                for j in range(GT):
                    t = t0 + j
                    nc.vector.max(maxv[:, t, :], e[:, j, :])
                    nc.vector.max_index(best8[:, t, :], maxv[:, t, :],
                                        e[:, j, :])

                if g == NG - 1 and (g + 1) % (NG // NPIECE) == 0:
                    # last group: decode (-> res output) before the divide
                    # chain so the small result isn't queued behind it
                    _decode_piece(nc, dec, best8, (g + 1) // (NG // NPIECE) - 1)
                    res30 = decp.tile([128, PRED], I32, tag="res30")
                    nc.vector.tensor_reduce(
                        res30[:],
                        dec["res30h"][:].rearrange("p q r -> p r q"),
                        axis=AX.X, op=ALU.max)
                    resf = decp.tile([128, PRED], I32, tag="resf")
                    nc.vector.tensor_scalar(resf[:], res30[:], -1, None,
                                            op0=ALU.add)
                    nc.sync.dma_start(res_d[:], resf[:])

                sums = smallp.tile([128, GT], F32, tag="sums")
                nc.vector.tensor_reduce(sums[:], e[:], axis=AX.X, op=ALU.add)
                rec = smallp.tile([128, GT], F32, tag="rec")
                nc.vector.reciprocal(rec[:], sums[:])
                probs = probsp.tile([128, GT, C], F32, tag="probs")
                for j in range(GT):
                    nc.vector.tensor_scalar(probs[:, j, :], e[:, j, :],
                                            rec[:, j:j + 1], None, op0=ALU.mult)
                nc.sync.dma_start(logits_d[:, t0:t0 + GT, :], probs[:])

                if g != NG - 1 and (g + 1) % (NG // NPIECE) == 0:
                    _decode_piece(nc, dec, best8, (g + 1) // (NG // NPIECE) - 1)



    nc.compile()
    return nc


def _get_nc(b_nonzero: bool):
    if b_nonzero not in _NC_CACHE:
        _NC_CACHE[b_nonzero] = _build(b_nonzero)
    return _NC_CACHE[b_nonzero]


def kernel(feat_vec, W, b, y=None, times=None, **_unused):
    global LAST_RESULT
    feat = np.ascontiguousarray(np.asarray(feat_vec, dtype=np.float32))
    W32 = np.ascontiguousarray(np.asarray(W, dtype=np.float32))
    b32 = np.asarray(b, dtype=np.float32).reshape(-1)
    assert feat.shape == (B_FULL, T, D), feat.shape
    assert W32.shape == (D, C), W32.shape

    b_nonzero = bool(np.any(b32))
    nc = _get_nc(b_nonzero)

    eye = np.eye(128, dtype=np.float32)
    in_maps = []
    for i in range(NCORES):
        m = {"feat": feat[i * BS:(i + 1) * BS], "w": W32, "ident": eye}
        if b_nonzero:
            m["eb"] = np.exp(b32).reshape(1, C).astype(np.float32)
        in_maps.append(m)

    res = run_bass_kernel_spmd(nc, in_maps, core_ids=list(range(NCORES)),
                               trace=TRACE, **TRACE_KWARGS)
    LAST_RESULT = res
    logits = np.concatenate([r["logits"] for r in res.results], axis=0)
    labs = np.concatenate([r["res"] for r in res.results], axis=0)
    return logits, labs


# revision 28
# speedup vs baseline: 1.0031x; 1.0031x over previous
"""CTC model kernel for Trainium2 (8 NeuronCores, pure data parallel).

reference computation:
    logits = softmax(feat_vec @ W + b)          # [1024, 80, 78]
    best   = argmax(logits, -1)                 # [1024, 80]
    res    = ctc_greedy_decode(best)            # [1024, 30] int32 (pad -1)
    return logits, res

Sharding: batch 1024 -> 8 cores x 128 rows. W/b replicated. Decode is
per-row so it shards with batch.

Per-core dataflow (B=128, T=80, D=512, C=78):
  16 groups of 5 timesteps:
    DMA feat[:, 5g:5g+5, :]  (10KB/partition, contiguous)
    per t: 4x PE transpose [128b,128d]->[128d,128b] (PSUM),
           PSUM->SBUF copy (ACT/DVE), 4x fp32 matmul vs W chunks -> PSUM
    ACT exp -> SBUF; DVE row-sums + reciprocal + normalize -> DMA out
    per t: DVE max8 + max_index -> best indices (argmax)
  decode tail (DVE): collapse repeats / drop blanks via prefix-scan cumsum
  and a one-hot gather, -> [128, 30] int32 -> DMA out.
"""
import sys

sys.path.insert(0, "/opt/trn_rl_repo")

import numpy as np

import concourse.bass as bass  # noqa: F401  (engine namespaces)
import concourse.mybir as mybir
import concourse.tile as tile
from concourse import bacc
from concourse.bass_utils import run_bass_kernel_spmd

F32 = mybir.dt.float32
I32 = mybir.dt.int32
I16 = mybir.dt.int16
U32 = mybir.dt.uint32
AX = mybir.AxisListType
ALU = mybir.AluOpType
ACTF = mybir.ActivationFunctionType

B_FULL, T, D, C = 1024, 80, 512, 78
NCORES = 8
BS = B_FULL // NCORES          # 128 rows per core
GT = 5                         # timesteps per group
NG = T // GT                   # 16 groups
NCH = D // 128                 # 4 contraction chunks
PRED = 30
BLANK = 77

TRACE = False                  # test.py sets kernel.TRACE = True to profile
TRACE_KWARGS = {}
LAST_RESULT = None

_NC_CACHE = {}

QT = T // 4  # decode piece length
NPIECE = 4


def _decode_piece(nc, dec, best8, q):
    """CTC decode for timesteps [q*QT, (q+1)*QT) — all int32 on DVE.
    Pieces 0..2 run mid-kernel (hidden under later groups); piece 3 is the
    only decode work on the tail. Cumsum state chains across pieces."""
    ALU = mybir.AluOpType
    AX = mybir.AxisListType
    ta, tb = q * QT, (q + 1) * QT
    best_i, prevb, keep, cum, posm, zeros = (
        dec["best_i"], dec["prevb"], dec["keep"], dec["cum"], dec["posm"],
        dec["zeros"])
    nc.vector.tensor_copy(best_i[:, ta:tb], best8[:, ta:tb, 0].bitcast(I32))
    if q == 0:
        nc.vector.memset(prevb[:, 0:1], -1)
        nc.vector.tensor_copy(prevb[:, 1:tb], best_i[:, 0:tb - 1])
    else:
        nc.vector.tensor_copy(prevb[:, ta:tb], best_i[:, ta - 1:tb - 1])
    n1 = dec["n1"]
    nc.vector.tensor_tensor(n1[:, ta:tb], best_i[:, ta:tb], prevb[:, ta:tb],
                            op=ALU.not_equal)
    n2 = dec["n2"]
    nc.vector.tensor_scalar(n2[:, ta:tb], best_i[:, ta:tb], BLANK, None,
                            op0=ALU.not_equal)
    nc.vector.tensor_tensor(keep[:, ta:tb], n1[:, ta:tb], n2[:, ta:tb],
                            op=ALU.mult)
    init = 0.0 if q == 0 else cum[:, ta - 1:ta]
    nc.vector.tensor_tensor_scan(cum[:, ta:tb], keep[:, ta:tb],
                                 zeros[:, ta:tb], init,
                                 op0=ALU.add, op1=ALU.add)
    pos = dec["pos"]
    nc.vector.tensor_scalar(pos[:, ta:tb], cum[:, ta:tb], -1, None,
                            op0=ALU.add)
    nc.vector.memset(posm[:, ta:tb], 999)
    nc.vector.copy_predicated(posm[:, ta:tb], keep[:, ta:tb], pos[:, ta:tb])
    bestp1 = dec["bestp1"]
    nc.vector.tensor_scalar(bestp1[:, ta:tb], best_i[:, ta:tb], 1, None,
                            op0=ALU.add)

    iota3v = dec["iota3"][:].rearrange("p (a b) -> p a b", a=PRED)
    m3v = dec["mask3"][:].rearrange("p (a b) -> p a b", a=PRED)
    nc.vector.tensor_tensor(
        m3v,
        posm[:, ta:tb].unsqueeze(1).broadcast_to([128, PRED, QT]),
        iota3v[:, :, ta:tb],
        op=ALU.is_equal)
    t3v = dec["tokm"][:].rearrange("p (a b) -> p a b", a=PRED)
    nc.vector.tensor_tensor(
        t3v, m3v,
        bestp1[:, ta:tb].unsqueeze(1).broadcast_to([128, PRED, QT]),
        op=ALU.mult)
    nc.vector.tensor_reduce(dec["res30h"][:, q, :], t3v, axis=AX.X,
                            op=ALU.max)


def _build(b_nonzero: bool):
    nc = bacc.Bacc(None, target_bir_lowering=False)

    feat_d = nc.dram_tensor("feat", [BS, T, D], F32, kind="ExternalInput")
    w_d = nc.dram_tensor("w", [D, C], F32, kind="ExternalInput")
    id_d = nc.dram_tensor("ident", [128, 128], F32, kind="ExternalInput")
    if b_nonzero:
        eb_d = nc.dram_tensor("eb", [1, C], F32, kind="ExternalInput")
    logits_d = nc.dram_tensor("logits", [BS, T, C], F32, kind="ExternalOutput")
    res_d = nc.dram_tensor("res", [BS, PRED], I32, kind="ExternalOutput")

    with tile.TileContext(nc) as tc:
        with tc.tile_pool(name="const", bufs=1) as constp, \
             tc.tile_pool(name="featp", bufs=4) as featp, \
             tc.tile_pool(name="ftsp", bufs=6) as ftsp, \
             tc.tile_pool(name="ep", bufs=3) as ep, \
             tc.tile_pool(name="probsp", bufs=2) as probsp, \
             tc.tile_pool(name="smallp", bufs=2) as smallp, \
             tc.tile_pool(name="argp", bufs=1) as argp, \
             tc.tile_pool(name="decp", bufs=1) as decp, \
             tc.tile_pool(name="ftpp", bufs=4, space="PSUM") as ftpp, \
             tc.tile_pool(name="lgp", bufs=3, space="PSUM") as lgp:

            # ---- constants (scalar-engine DGE queue: overlaps feat triggers) ----
            id_s = constp.tile([128, 128], F32, tag="id_s")
            nc.sync.dma_start(id_s[:], id_d[:])
            w_s = constp.tile([128, NCH, C], F32, tag="w_s")
            nc.scalar.dma_start(w_s[:], w_d[:].rearrange("(c p) n -> p c n", p=128))
            if b_nonzero:
                eb_s = constp.tile([128, C], F32, tag="eb_s")
                nc.scalar.dma_start(eb_s[:], eb_d[:].partition_broadcast(128))

            # iota over the middle axis of [128, PRED, T]: value = j
            iota3 = decp.tile([128, PRED * T], I16, tag="iota3")
            nc.gpsimd.iota(iota3[:].rearrange("p (a b) -> p a b", a=PRED),
                           pattern=[[1, PRED], [0, T]], base=0,
                           channel_multiplier=0)

            # persistent decode state ([128, T] int32 each)
            dec = {"iota3": iota3}
            for nm in ["best_i", "prevb", "keep", "cum", "posm", "zeros",
                       "n1", "n2", "pos", "bestp1"]:
                dec[nm] = decp.tile([128, T], I16, tag=f"dec_{nm}", name=f"dec_{nm}")
            dec["mask3"] = decp.tile([128, PRED * QT], I16, tag="dec_mask3", name="dec_mask3")
            dec["tokm"] = decp.tile([128, PRED * QT], I16, tag="dec_tokm", name="dec_tokm")
            dec["res30h"] = decp.tile([128, NPIECE, PRED], I16, tag="dec_res30h", name="dec_res30h")
            nc.vector.memset(dec["zeros"][:], 0)

            # per-t argmax results (free layout [t, 8])
            maxv = argp.tile([128, T, 8], F32, tag="maxv")
            best8 = argp.tile([128, T, 8], U32, tag="best8")

            # HAM warm-up: dummy transposes on a memset tile (available ~6us
            # before the DMA'd identity) while the first feat chunks land.
            # PE runs at 1.2 GHz until ~3.4us of sustained activity.
            warm_in = constp.tile([128, 128], F32, tag="warm_in")
            nc.vector.memset(warm_in[:], 0.0)
            warmp = ftpp.tile([128, 128], F32, tag="warm", name="warm", bufs=1)
            for _ in range(40):
                nc.tensor.transpose(warmp[:], warm_in[:], warm_in[:])

            # ---- main loop over groups of GT timesteps ----
            for g in range(NG):
                t0 = g * GT
                ftile = featp.tile([128, GT, D], F32, tag="ftile")
                if g == 0:
                    # split the first load so PE can start ~3us earlier;
                    # first 128-col chunk smallest so transpose #1 unblocks fast
                    nc.sync.dma_start(ftile[:, 0, 0:128], feat_d[:, 0, 0:128])
                    nc.sync.dma_start(ftile[:, 0, 128:D], feat_d[:, 0, 128:D])
                    for j in range(1, GT):
                        nc.sync.dma_start(ftile[:, j, :], feat_d[:, j, :])
                else:
                    nc.sync.dma_start(ftile[:], feat_d[:, t0:t0 + GT, :])

                lg = lgp.tile([128, GT, C], F32, tag="lg")
                for j in range(GT):
                    ftp = ftpp.tile([128, D], F32, tag="ftp")
                    for c in range(NCH):
                        nc.tensor.transpose(
                            ftp[:, c * 128:(c + 1) * 128],
                            ftile[:, j, c * 128:(c + 1) * 128],
                            id_s[:])
                    fts = ftsp.tile([128, D], F32, tag="fts")
                    nc.scalar.copy(fts[:], ftp[:])
                    for c in range(NCH):
                        nc.tensor.matmul(
                            lg[:, j, :],
                            fts[:, c * 128:(c + 1) * 128],
                            w_s[:, c, :],
                            start=(c == 0), stop=(c == NCH - 1))

                e = ep.tile([128, GT, C], F32, tag="e")
                if g == NG - 1:
                    # split exp per-t so the final argmax chain starts sooner
                    for j in reversed(range(GT)):
                        nc.scalar.activation(e[:, j, :], lg[:, j, :], ACTF.Exp)
                else:
                    nc.scalar.activation(e[:], lg[:], ACTF.Exp)
                if b_nonzero:
                    nc.vector.tensor_tensor(
                        e[:], e[:],
                        eb_s[:].unsqueeze(1).broadcast_to([128, GT, C]),
                        op=ALU.mult)

                # argmax first: it gates the decode tail
                for j in range(GT):
                    t = t0 + j
                    nc.vector.max(maxv[:, t, :], e[:, j, :])
                    nc.vector.max_index(best8[:, t, :], maxv[:, t, :],
                                        e[:, j, :])

                if g == NG - 1 and (g + 1) % (NG // NPIECE) == 0:
                    # last group: decode (-> res output) before the divide
                    # chain so the small result isn't queued behind it
                    _decode_piece(nc, dec, best8, (g + 1) // (NG // NPIECE) - 1)
                    res30 = decp.tile([128, PRED], I32, tag="res30")
                    nc.vector.tensor_reduce(
                        res30[:],
                        dec["res30h"][:].rearrange("p q r -> p r q"),
                        axis=AX.X, op=ALU.max)
                    resf = decp.tile([128, PRED], I32, tag="resf")
                    nc.vector.tensor_scalar(resf[:], res30[:], -1, None,
                                            op0=ALU.add)
                    nc.sync.dma_start(res_d[:], resf[:])

                sums = smallp.tile([128, GT], F32, tag="sums")
                nc.vector.tensor_reduce(sums[:], e[:], axis=AX.X, op=ALU.add)
                rec = smallp.tile([128, GT], F32, tag="rec")
                nc.vector.reciprocal(rec[:], sums[:])
                probs = probsp.tile([128, GT, C], F32, tag="probs")
                for j in range(GT):
                    nc.vector.tensor_scalar(probs[:, j, :], e[:, j, :],
                                            rec[:, j:j + 1], None, op0=ALU.mult)
                nc.sync.dma_start(logits_d[:, t0:t0 + GT, :], probs[:])

                if g != NG - 1 and (g + 1) % (NG // NPIECE) == 0:
                    _decode_piece(nc, dec, best8, (g + 1) // (NG // NPIECE) - 1)



    nc.compile()
    return nc


def _get_nc(b_nonzero: bool):
    if b_nonzero not in _NC_CACHE:
        _NC_CACHE[b_nonzero] = _build(b_nonzero)
    return _NC_CACHE[b_nonzero]


def kernel(feat_vec, W, b, y=None, times=None, **_unused):
    global LAST_RESULT
    feat = np.ascontiguousarray(np.asarray(feat_vec, dtype=np.float32))
    W32 = np.ascontiguousarray(np.asarray(W, dtype=np.float32))
    b32 = np.asarray(b, dtype=np.float32).reshape(-1)
    assert feat.shape == (B_FULL, T, D), feat.shape
    assert W32.shape == (D, C), W32.shape

    b_nonzero = bool(np.any(b32))
    nc = _get_nc(b_nonzero)

    eye = np.eye(128, dtype=np.float32)
    in_maps = []
    for i in range(NCORES):
        m = {"feat": feat[i * BS:(i + 1) * BS], "w": W32, "ident": eye}
        if b_nonzero:
            m["eb"] = np.exp(b32).reshape(1, C).astype(np.float32)
        in_maps.append(m)

    res = run_bass_kernel_spmd(nc, in_maps, core_ids=list(range(NCORES)),
                               trace=TRACE, **TRACE_KWARGS)
    LAST_RESULT = res
    logits = np.concatenate([r["logits"] for r in res.results], axis=0)
    labs = np.concatenate([r["res"] for r in res.results], axis=0)
    return logits, labs
